# revision 13
# baseline (speedup 1.0000x reference)
"""Trainium2 Bass kernel for nn_DCT_Features (dense_cnn).

Math: everything before the LeakyReLU is linear, so the whole module
(3D DCT-II -> mean over dct bins -> per-subwindow full-volume Conv3d)
collapses to one GEMM per subwindow:

  out[b, s*128+k] = LeakyReLU( sum_{n,phi} x[b, s, n, phi] * W[s, phi, k]
                               + conv_b[s, k] )

with W[s, (t,h,w), k] = 0.5 * sum_{f,g,j} conv_w[s,k,f,g,j] Ct[f,t] Ch[g,h] Cw[j,w]
(the mean's 1/2 folded in; the bin-sum is folded into the matmul by doubling
K to 4096 and reusing the same weight tile for both bins).

Precision strategy (rel-err budget 2e-2): x is quantized host-side to
fp8 e3m4 (exactly measured 1.41e-2 max rel err on the reference inputs),
weights stay bf16, PSUM accumulates fp32. This halves PE time vs bf16
(1 cycle/row with fp8 moving operand) and halves the dominant x DMA
traffic to 4 MiB/core.

Sharding: pure data parallel over batch, 8 cores x 512 rows; W/bias
replicated. Host-side marshaling lays each core's shard out feature-major
([s, n, kt, p, b]) so every DMA is contiguous with >=512B runs (full
360 GB/s descriptor rate). Per core:

  DMA x tiles -> fp8 matmul accumulate (kout on partitions, batch on free,
  K=4096 per subwindow; PSUM seeded with the bias via a K=1 bf16 matmul
  of bias-row x ones-row) -> 2-op exact LeakyReLU on DVE -> DMA out in
  bf16 (host upcasts + un-transposes while gathering the 8 shards).
"""

import os
from contextlib import ExitStack

import numpy as np
import ml_dtypes

import concourse.bass as bass
import concourse.tile as tile
from concourse import bacc, mybir
from concourse.bass_utils import run_bass_kernel_spmd

# Static problem config (hardcoded per contract)
B_FULL = 4096
N_CORES = 8
B_CORE = B_FULL // N_CORES      # 512 batch rows per core
N_SW = 2                        # subwindows
DCT_NBINS = 2
NDCT = 32                       # freqs per subwindow
H = W = 8
KF = NDCT * H * W               # 2048 contraction dim per (subwindow, bin)
KT = KF // 128                  # 16 k-tiles per (subwindow, bin)
NT = N_SW * DCT_NBINS * KT      # 64 total k-tiles in stream order (s, n, kt)
KOUT = 128                      # output channels per subwindow
SLOPE = 0.001

_CACHE = {}
LAST_RESULT = None


def _dct_mat(N):
    n = np.arange(N)
    k = np.arange(N)[:, None]
    return 2.0 * np.cos(np.pi * (2 * n + 1) * k / (2 * N))  # [k, n], float64


def _fold_weights(conv_w, conv_b):
    """Fold DCT matrices + mean into the conv weights (float64 host math)."""
    cw = np.asarray(conv_w, np.float64)          # [s, k, f, g, j]
    Ct = _dct_mat(NDCT)                          # [f, t]
    Ch = _dct_mat(H)                             # [g, h]
    Cw = _dct_mat(W)                             # [j, w]
    we = np.einsum("skfgj,ft,gh,jw->sthwk", cw, Ct, Ch, Cw) * 0.5
    we = we.reshape(N_SW, KF, KOUT)              # [s, phi, k]
    # SBUF layout: w_sb[p, (s*KT+kt)*128 + k] = we[s, kt*128+p, k]
    w_host = (
        we.reshape(N_SW, KT, 128, KOUT).transpose(2, 0, 1, 3).reshape(128, N_SW * KT * KOUT)
    ).astype(ml_dtypes.bfloat16)
    # bias + ones row for the PSUM-seeding K=1 matmul: [1, 2*KOUT + B_CORE]
    bvec = np.zeros((1, N_SW * KOUT + B_CORE), ml_dtypes.bfloat16)
    bvec[0, : N_SW * KOUT] = (
        np.asarray(conv_b, np.float64).reshape(N_SW * KOUT).astype(ml_dtypes.bfloat16)
    )
    bvec[0, N_SW * KOUT :] = 1.0
    return np.ascontiguousarray(w_host), bvec


def _shard_x(x):
    """Marshal x into per-core feature-major fp8 tiles.

    Returns per-core arrays of shape [NT*128, B_CORE] (fp8 e3m4) where
    row ((s*2+n)*KT+kt)*128+p, column b holds x[c*B_CORE+b, f] with
    f = s*4096 + n*2048 + kt*128 + p.
    """
    X = np.asarray(x, np.float32).reshape(B_FULL, N_SW * DCT_NBINS * KF)
    Xq = X.astype(ml_dtypes.float8_e3m4)
    shards = []
    for c in range(N_CORES):
        v = Xq[c * B_CORE : (c + 1) * B_CORE].reshape(B_CORE, N_SW, DCT_NBINS, KT, 128)
        p = v.transpose(1, 2, 3, 4, 0)  # [s, n, kt, p, b]
        shards.append(np.ascontiguousarray(p).reshape(NT * 128, B_CORE))
    return shards


def _chunk_plan():
    """(t_start, n_t) DMA chunks over the 64-tile stream. Tiny chunks at the
    front (earlier first matmul), 8-tile chunks in the middle (fewer HWDGE
    preps -> no DMA-engine gaps), short taper at the tail."""
    plan = [(0, 1), (1, 1), (2, 2), (4, 4)]
    t = 8
    while t < 56:
        plan.append((t, 8))
        t += 8
    plan += [(56, 4), (60, 2), (62, 1), (63, 1)]
    return plan


def _build_program():
    nc = bacc.Bacc("TRN2", target_bir_lowering=False, debug=False, num_devices=N_CORES)
    f32 = mybir.dt.float32
    bf16 = mybir.dt.bfloat16
    f8 = mybir.dt.float8e3

    x_ap = nc.dram_tensor("x", [NT * 128, B_CORE], f8, kind="ExternalInput").ap()
    w_ap = nc.dram_tensor("w", [128, N_SW * KT * KOUT], bf16, kind="ExternalInput").ap()
    b_ap = nc.dram_tensor("bvec", [1, N_SW * KOUT + B_CORE], bf16, kind="ExternalInput").ap()
    # output stays transposed [s*128+k, b] in bf16; host upcasts + un-transposes
    out_ap = nc.dram_tensor("out", [N_SW * KOUT, B_CORE], bf16, kind="ExternalOutput").ap()

    N_WARMUP = 9

    with tile.TileContext(nc) as tc, ExitStack() as ctx:
        const = ctx.enter_context(tc.tile_pool(name="const", bufs=1))
        x_pool = ctx.enter_context(tc.tile_pool(name="xp", bufs=6))
        osb_pool = ctx.enter_context(tc.tile_pool(name="osb", bufs=4))
        pout_pool = ctx.enter_context(tc.tile_pool(name="pout", bufs=1, space="PSUM"))

        # ones row for PE warm-up matmuls (burn the p-state ramp while the
        # first DMAs are in flight); small memset so it's ready ~1.2us in
        ones_sb = const.tile([1, 384], bf16)
        nc.vector.memset(ones_sb[:], 1.0)

        bvec_sb = const.tile([1, N_SW * KOUT + B_CORE], bf16)
        nc.gpsimd.dma_start(out=bvec_sb[:], in_=b_ap[:])

        x_re = x_ap.rearrange("(t p) b -> p t b", p=128)  # [128, 64, 512]

        # weights in SEPARATE tiles per chunk: readers of one chunk must not
        # wait on DMAs of the others (tile-granularity dependency tracking).
        # kt0 alone so the very first matmul starts early; s1's half via Pool
        # SWDGE (needed only ~60% into the stream).
        w0_sb = const.tile([128, 128], bf16)
        w1_sb = const.tile([128, 896], bf16)
        w2_sb = const.tile([128, 1024], bf16)
        w3_sb = const.tile([128, 2048], bf16)
        nc.gpsimd.dma_start(out=w3_sb[:], in_=w_ap[:, 2048:4096])

        def w_tile(s, kt):
            if s == 1:
                return w3_sb[:, bass.ts(kt, 128)]
            if kt == 0:
                return w0_sb[:]
            if kt < 8:
                return w1_sb[:, bass.ts(kt - 1, 128)]
            return w2_sb[:, bass.ts(kt - 8, 128)]

        psums = {}
        for s in range(N_SW):
            psums[s] = pout_pool.tile([KOUT, B_CORE], f32, name=f"psum_{s}")
        scratch = pout_pool.tile([KOUT, B_CORE], f32, name="psum_warm")

        for i in range(N_WARMUP):
            nc.tensor.matmul(
                scratch[:, 0:256],
                lhsT=ones_sb[0:1, 0:KOUT],
                rhs=ones_sb[0:1, KOUT:],
                start=True,
                stop=True,
            )

        def issue_epilogue(s):
            # exact LeakyReLU (bias already seeded in PSUM), obeying the
            # one-PSUM-operand-per-instruction HW rule:
            #   r   = (psum max 0) * (1-SLOPE)         [dual-op tensor_scalar]
            #   out = (psum * SLOPE) + r               [scalar_tensor_tensor]
            r = osb_pool.tile([KOUT, B_CORE], f32, tag="r", name=f"r_{s}")
            nc.vector.tensor_scalar(
                r[:],
                psums[s][:],
                0.0,
                1.0 - SLOPE,
                mybir.AluOpType.max,
                mybir.AluOpType.mult,
            )
            osb = osb_pool.tile([KOUT, B_CORE], bf16, tag="osb", name=f"osb_{s}")
            nc.vector.scalar_tensor_tensor(
                osb[:],
                psums[s][:],
                SLOPE,
                r[:],
                mybir.AluOpType.mult,
                mybir.AluOpType.add,
            )
            # s0's out mid-stream via Pool SWDGE (keeps SP/ACT free for x);
            # s1's out at the tail via SP (idle by then, fastest DGE)
            eng = nc.gpsimd if s == 0 else nc.sync
            eng.dma_start(out=out_ap[bass.ts(s, KOUT), :], in_=osb[:])

        chunks = _chunk_plan()
        for g, (t0, nt) in enumerate(chunks):
            xab = x_pool.tile([128, 8, B_CORE], f8, tag="xab", name=f"xab_{g}")
            dma_eng = nc.sync if g % 2 == 0 else nc.scalar
            dma_eng.dma_start(out=xab[:, 0:nt, :], in_=x_re[:, bass.ds(t0, nt), :])
            if t0 == 0:
                # w kt0 right behind the first x tile on the other queue
                nc.scalar.dma_start(out=w0_sb[:], in_=w_ap[:, 0:128])
            elif t0 == 1:
                # rest of s0's weights ride the fast queues in use order
                nc.scalar.dma_start(out=w1_sb[:], in_=w_ap[:, 128:1024])
            elif t0 == 2:
                nc.sync.dma_start(out=w2_sb[:], in_=w_ap[:, 1024:2048])
            for j in range(nt):
                t = t0 + j
                s, n, kt = t // (DCT_NBINS * KT), (t // KT) % DCT_NBINS, t % KT
                nc.tensor.matmul(
                    psums[s][:],
                    lhsT=w_tile(s, kt),
                    rhs=xab[:, j, :],
                    start=(n == 0 and kt == 0),
                    stop=(n == DCT_NBINS - 1 and kt == KT - 1),
                )
                if n == 0 and kt == 7:
                    # bias seeding rides mid-group (any accumulation order
                    # works); bvec arrives ~3us in via Pool SWDGE
                    nc.tensor.matmul(
                        psums[s][:],
                        lhsT=bvec_sb[0:1, bass.ts(s, KOUT)],
                        rhs=bvec_sb[0:1, N_SW * KOUT :],
                        start=False,
                        stop=False,
                    )
                if n == DCT_NBINS - 1 and kt == KT - 1:
                    issue_epilogue(s)

    nc.compile()
    return nc


def _get_program():
    if "nc" not in _CACHE:
        _CACHE["nc"] = _build_program()
    return _CACHE["nc"]


def kernel(x, conv_w, conv_b):
    global LAST_RESULT
    shards = _shard_x(x)
    w_host, bvec = _fold_weights(conv_w, conv_b)

    nc = _get_program()
    in_maps = [{"x": shards[c], "w": w_host, "bvec": bvec} for c in range(N_CORES)]
    trace = bool(int(os.environ.get("DCT_TRACE", "0")))
    res = run_bass_kernel_spmd(nc, in_maps, list(range(N_CORES)), trace=trace)
    LAST_RESULT = res
    # per-core output is [s*128+k, b] bf16; upcast + un-transpose during gather
    out = np.concatenate(
        [
            np.ascontiguousarray(res.results[c]["out"].astype(np.float32).T)
            for c in range(N_CORES)
        ],
        axis=0,
    )
    return out


# revision 18
# speedup vs baseline: 1.0599x; 1.0599x over previous
"""Trainium2 Bass kernel for nn_DCT_Features (dense_cnn).

Math: everything before the LeakyReLU is linear, so the whole module
(3D DCT-II -> mean over dct bins -> per-subwindow full-volume Conv3d)
collapses to one GEMM per subwindow:

  out[b, s*128+k] = LeakyReLU( sum_{n,phi} x[b, s, n, phi] * W[s, phi, k]
                               + conv_b[s, k] )

with W[s, (t,h,w), k] = 0.5 * sum_{f,g,j} conv_w[s,k,f,g,j] Ct[f,t] Ch[g,h] Cw[j,w]
(the mean's 1/2 folded in; the bin-sum is folded into the matmul by doubling
K to 4096 and reusing the same weight tile for both bins).

Precision strategy (rel-err budget 2e-2): x is quantized host-side to
fp8 e3m4 (exactly measured 1.41e-2 max rel err on the reference inputs),
weights stay bf16, PSUM accumulates fp32. This halves PE time vs bf16
(1 cycle/row with fp8 moving operand) and halves the dominant x DMA
traffic to 4 MiB/core.

Sharding: pure data parallel over batch, 8 cores x 512 rows; W/bias
replicated. Host-side marshaling lays each core's shard out feature-major
([s, n, kt, p, b]) so every DMA is contiguous with >=512B runs (full
360 GB/s descriptor rate). Per core:

  DMA x tiles -> fp8 matmul accumulate (kout on partitions, batch on free,
  K=4096 per subwindow; PSUM seeded with the bias via a K=1 bf16 matmul
  of bias-row x ones-row) -> 2-op exact LeakyReLU on DVE -> DMA out in
  bf16 (host upcasts + un-transposes while gathering the 8 shards).
"""

import os
from contextlib import ExitStack

import numpy as np
import ml_dtypes

import concourse.bass as bass
import concourse.tile as tile
from concourse import bacc, mybir
from concourse.bass_utils import run_bass_kernel_spmd

# Static problem config (hardcoded per contract)
B_FULL = 4096
N_CORES = 8
B_CORE = B_FULL // N_CORES      # 512 batch rows per core
N_SW = 2                        # subwindows
DCT_NBINS = 2
NDCT = 32                       # freqs per subwindow
H = W = 8
KF = NDCT * H * W               # 2048 contraction dim per (subwindow, bin)
KT = KF // 128                  # 16 k-tiles per (subwindow, bin)
NT = N_SW * DCT_NBINS * KT      # 64 total k-tiles in stream order (s, n, kt)
KOUT = 128                      # output channels per subwindow
SLOPE = 0.001

_CACHE = {}
LAST_RESULT = None


def _dct_mat(N):
    n = np.arange(N)
    k = np.arange(N)[:, None]
    return 2.0 * np.cos(np.pi * (2 * n + 1) * k / (2 * N))  # [k, n], float64


def _fold_weights(conv_w, conv_b):
    """Fold DCT matrices + mean into the conv weights (float64 host math)."""
    cw = np.asarray(conv_w, np.float64)          # [s, k, f, g, j]
    Ct = _dct_mat(NDCT)                          # [f, t]
    Ch = _dct_mat(H)                             # [g, h]
    Cw = _dct_mat(W)                             # [j, w]
    we = np.einsum("skfgj,ft,gh,jw->sthwk", cw, Ct, Ch, Cw) * 0.5
    we = we.reshape(N_SW, KF, KOUT)              # [s, phi, k]
    # SBUF layout: w_sb[p, (s*KT+kt)*128 + k] = we[s, kt*128+p, k]
    w_host = (
        we.reshape(N_SW, KT, 128, KOUT).transpose(2, 0, 1, 3).reshape(128, N_SW * KT * KOUT)
    ).astype(ml_dtypes.bfloat16)
    # bias + ones row for the PSUM-seeding K=1 matmul: [1, 2*KOUT + B_CORE]
    bvec = np.zeros((1, N_SW * KOUT + B_CORE), ml_dtypes.bfloat16)
    bvec[0, : N_SW * KOUT] = (
        np.asarray(conv_b, np.float64).reshape(N_SW * KOUT).astype(ml_dtypes.bfloat16)
    )
    bvec[0, N_SW * KOUT :] = 1.0
    return np.ascontiguousarray(w_host), bvec


def _shard_x(x):
    """Marshal x into per-core feature-major fp8 tiles.

    Returns per-core arrays of shape [NT*128, B_CORE] (fp8 e3m4) where
    row ((s*2+n)*KT+kt)*128+p, column b holds x[c*B_CORE+b, f] with
    f = s*4096 + n*2048 + kt*128 + p.
    """
    X = np.asarray(x, np.float32).reshape(B_FULL, N_SW * DCT_NBINS * KF)
    Xq = X.astype(ml_dtypes.float8_e3m4)
    shards = []
    for c in range(N_CORES):
        v = Xq[c * B_CORE : (c + 1) * B_CORE].reshape(B_CORE, N_SW, DCT_NBINS, KT, 128)
        p = v.transpose(1, 2, 3, 4, 0)  # [s, n, kt, p, b]
        shards.append(np.ascontiguousarray(p).reshape(NT * 128, B_CORE))
    return shards


def _chunk_plan():
    """(t_start, n_t) DMA chunks over the 64-tile stream. Small chunks at the
    front (early first matmul), 4-tile chunks through the middle, short taper
    at the tail (less serial work after the final DMA)."""
    plan = [(0, 2), (2, 2)]
    plan += [(t, 4) for t in range(4, 60, 4)]
    plan += [(60, 2), (62, 1), (63, 1)]
    return plan


def _build_program():
    nc = bacc.Bacc("TRN2", target_bir_lowering=False, debug=False, num_devices=N_CORES)
    f32 = mybir.dt.float32
    bf16 = mybir.dt.bfloat16
    f8 = mybir.dt.float8e3

    x_ap = nc.dram_tensor("x", [NT * 128, B_CORE], f8, kind="ExternalInput").ap()
    w_ap = nc.dram_tensor("w", [128, N_SW * KT * KOUT], bf16, kind="ExternalInput").ap()
    b_ap = nc.dram_tensor("bvec", [1, N_SW * KOUT + B_CORE], bf16, kind="ExternalInput").ap()
    # output stays transposed [s*128+k, b] in bf16; host upcasts + un-transposes
    out_ap = nc.dram_tensor("out", [N_SW * KOUT, B_CORE], bf16, kind="ExternalOutput").ap()

    N_WARMUP = 9

    with tile.TileContext(nc) as tc, ExitStack() as ctx:
        const = ctx.enter_context(tc.tile_pool(name="const", bufs=1))
        x_pool = ctx.enter_context(tc.tile_pool(name="xp", bufs=6))
        osb_pool = ctx.enter_context(tc.tile_pool(name="osb", bufs=4))
        pout_pool = ctx.enter_context(tc.tile_pool(name="pout", bufs=1, space="PSUM"))

        # ones row for PE warm-up matmuls (burn the p-state ramp while the
        # first DMAs are in flight); small memset so it's ready ~1.2us in
        ones_sb = const.tile([1, 384], bf16)
        nc.vector.memset(ones_sb[:], 1.0)

        x_re = x_ap.rearrange("(t p) b -> p t b", p=128)  # [128, 64, 512]

        # weights via Pool SWDGE (own descriptor engine -> keeps the HWDGE
        # queues free for the x stream), in SEPARATE tiles per chunk so
        # readers of one chunk never wait on DMAs of the others
        # (tile-granularity dependency tracking). Pool's ~1us serial preps
        # self-stagger the four transfers across the early stream; each lands
        # comfortably before its first consuming matmul. bvec last (needed
        # only by the mid-group bias matmul).
        wch = [const.tile([128, 1024], bf16, name=f"wch_{i}") for i in range(4)]
        for i in range(4):
            nc.gpsimd.dma_start(out=wch[i][:], in_=w_ap[:, bass.ts(i, 1024)])

        bvec_sb = const.tile([1, N_SW * KOUT + B_CORE], bf16)
        nc.gpsimd.dma_start(out=bvec_sb[:], in_=b_ap[:])

        def w_tile(s, kt):
            return wch[s * 2 + kt // 8][:, bass.ts(kt % 8, 128)]

        psums = {}
        for s in range(N_SW):
            psums[s] = pout_pool.tile([KOUT, B_CORE], f32, name=f"psum_{s}")
        scratch = pout_pool.tile([KOUT, B_CORE], f32, name="psum_warm")

        for i in range(N_WARMUP):
            nc.tensor.matmul(
                scratch[:, 0:256],
                lhsT=ones_sb[0:1, 0:KOUT],
                rhs=ones_sb[0:1, KOUT:],
                start=True,
                stop=True,
            )

        def issue_epilogue(s):
            # exact LeakyReLU (bias already seeded in PSUM), obeying the
            # one-PSUM-operand-per-instruction HW rule:
            #   r   = (psum max 0) * (1-SLOPE)         [dual-op tensor_scalar]
            #   out = (psum * SLOPE) + r               [scalar_tensor_tensor]
            r = osb_pool.tile([KOUT, B_CORE], f32, tag="r", name=f"r_{s}")
            nc.vector.tensor_scalar(
                r[:],
                psums[s][:],
                0.0,
                1.0 - SLOPE,
                mybir.AluOpType.max,
                mybir.AluOpType.mult,
            )
            osb = osb_pool.tile([KOUT, B_CORE], bf16, tag="osb", name=f"osb_{s}")
            nc.vector.scalar_tensor_tensor(
                osb[:],
                psums[s][:],
                SLOPE,
                r[:],
                mybir.AluOpType.mult,
                mybir.AluOpType.add,
            )
            # s0's out mid-stream via Pool SWDGE (keeps SP/ACT free for x);
            # s1's out at the tail via SP (idle by then, fastest DGE)
            eng = nc.gpsimd if s == 0 else nc.sync
            eng.dma_start(out=out_ap[bass.ts(s, KOUT), :], in_=osb[:])

        chunks = _chunk_plan()
        for g, (t0, nt) in enumerate(chunks):
            xab = x_pool.tile([128, 4, B_CORE], f8, tag="xab", name=f"xab_{g}")
            dma_eng = nc.sync if g % 2 == 0 else nc.scalar
            dma_eng.dma_start(out=xab[:, 0:nt, :], in_=x_re[:, bass.ds(t0, nt), :])
            for j in range(nt):
                t = t0 + j
                s, n, kt = t // (DCT_NBINS * KT), (t // KT) % DCT_NBINS, t % KT
                nc.tensor.matmul(
                    psums[s][:],
                    lhsT=w_tile(s, kt),
                    rhs=xab[:, j, :],
                    start=(n == 0 and kt == 0),
                    stop=(n == DCT_NBINS - 1 and kt == KT - 1),
                )
                if n == 0 and kt == 12:
                    # bias seeding rides mid-group (any accumulation order
                    # works); bvec arrives ~7us in via Pool SWDGE
                    nc.tensor.matmul(
                        psums[s][:],
                        lhsT=bvec_sb[0:1, bass.ts(s, KOUT)],
                        rhs=bvec_sb[0:1, N_SW * KOUT :],
                        start=False,
                        stop=False,
                    )
                if n == DCT_NBINS - 1 and kt == KT - 1:
                    issue_epilogue(s)

    nc.compile()
    return nc


def _get_program():
    if "nc" not in _CACHE:
        _CACHE["nc"] = _build_program()
    return _CACHE["nc"]


def kernel(x, conv_w, conv_b):
    global LAST_RESULT
    shards = _shard_x(x)
    w_host, bvec = _fold_weights(conv_w, conv_b)

    nc = _get_program()
    in_maps = [{"x": shards[c], "w": w_host, "bvec": bvec} for c in range(N_CORES)]
    trace = bool(int(os.environ.get("DCT_TRACE", "0")))
    res = run_bass_kernel_spmd(nc, in_maps, list(range(N_CORES)), trace=trace)
    LAST_RESULT = res
    # per-core output is [s*128+k, b] bf16; upcast + un-transpose during gather
    out = np.concatenate(
        [
            np.ascontiguousarray(res.results[c]["out"].astype(np.float32).T)
            for c in range(N_CORES)
        ],
        axis=0,
    )
    return out


# revision 19
# speedup vs baseline: 1.0684x; 1.0080x over previous
"""Trainium2 Bass kernel for nn_DCT_Features (dense_cnn).

Math: everything before the LeakyReLU is linear, so the whole module
(3D DCT-II -> mean over dct bins -> per-subwindow full-volume Conv3d)
collapses to one GEMM per subwindow:

  out[b, s*128+k] = LeakyReLU( sum_{n,phi} x[b, s, n, phi] * W[s, phi, k]
                               + conv_b[s, k] )

with W[s, (t,h,w), k] = 0.5 * sum_{f,g,j} conv_w[s,k,f,g,j] Ct[f,t] Ch[g,h] Cw[j,w]
(the mean's 1/2 folded in; the bin-sum is folded into the matmul by doubling
K to 4096 and reusing the same weight tile for both bins).

Precision strategy (rel-err budget 2e-2): x is quantized host-side to
fp8 e3m4 (exactly measured 1.41e-2 max rel err on the reference inputs),
weights stay bf16, PSUM accumulates fp32. This halves PE time vs bf16
(1 cycle/row with fp8 moving operand) and halves the dominant x DMA
traffic to 4 MiB/core.

Sharding: pure data parallel over batch, 8 cores x 512 rows; W/bias
replicated. Host-side marshaling lays each core's shard out feature-major
([s, n, kt, p, b]) so every DMA is contiguous with >=512B runs (full
360 GB/s descriptor rate). Per core:

  DMA x tiles -> fp8 matmul accumulate (kout on partitions, batch on free,
  K=4096 per subwindow; PSUM seeded with the bias via a K=1 bf16 matmul
  of bias-row x ones-row) -> 2-op exact LeakyReLU on DVE -> DMA out in
  bf16 (host upcasts + un-transposes while gathering the 8 shards).
"""

import os
from contextlib import ExitStack

import numpy as np
import ml_dtypes

import concourse.bass as bass
import concourse.tile as tile
from concourse import bacc, mybir
from concourse.bass_utils import run_bass_kernel_spmd

# Static problem config (hardcoded per contract)
B_FULL = 4096
N_CORES = 8
B_CORE = B_FULL // N_CORES      # 512 batch rows per core
N_SW = 2                        # subwindows
DCT_NBINS = 2
NDCT = 32                       # freqs per subwindow
H = W = 8
KF = NDCT * H * W               # 2048 contraction dim per (subwindow, bin)
KT = KF // 128                  # 16 k-tiles per (subwindow, bin)
NT = N_SW * DCT_NBINS * KT      # 64 total k-tiles in stream order (s, n, kt)
KOUT = 128                      # output channels per subwindow
SLOPE = 0.001

_CACHE = {}
LAST_RESULT = None


def _dct_mat(N):
    n = np.arange(N)
    k = np.arange(N)[:, None]
    return 2.0 * np.cos(np.pi * (2 * n + 1) * k / (2 * N))  # [k, n], float64


def _fold_weights(conv_w, conv_b):
    """Fold DCT matrices + mean into the conv weights (float64 host math)."""
    cw = np.asarray(conv_w, np.float64)          # [s, k, f, g, j]
    Ct = _dct_mat(NDCT)                          # [f, t]
    Ch = _dct_mat(H)                             # [g, h]
    Cw = _dct_mat(W)                             # [j, w]
    we = np.einsum("skfgj,ft,gh,jw->sthwk", cw, Ct, Ch, Cw) * 0.5
    we = we.reshape(N_SW, KF, KOUT)              # [s, phi, k]
    # SBUF layout: w_sb[p, (s*KT+kt)*128 + k] = we[s, kt*128+p, k]
    w_host = (
        we.reshape(N_SW, KT, 128, KOUT).transpose(2, 0, 1, 3).reshape(128, N_SW * KT * KOUT)
    ).astype(ml_dtypes.bfloat16)
    # bias + ones row for the PSUM-seeding K=1 matmul: [1, 2*KOUT + B_CORE]
    bvec = np.zeros((1, N_SW * KOUT + B_CORE), ml_dtypes.bfloat16)
    bvec[0, : N_SW * KOUT] = (
        np.asarray(conv_b, np.float64).reshape(N_SW * KOUT).astype(ml_dtypes.bfloat16)
    )
    bvec[0, N_SW * KOUT :] = 1.0
    return np.ascontiguousarray(w_host), bvec


def _shard_x(x):
    """Marshal x into per-core feature-major fp8 tiles.

    Returns per-core arrays of shape [NT*128, B_CORE] (fp8 e3m4) where
    row ((s*2+n)*KT+kt)*128+p, column b holds x[c*B_CORE+b, f] with
    f = s*4096 + n*2048 + kt*128 + p.
    """
    X = np.asarray(x, np.float32).reshape(B_FULL, N_SW * DCT_NBINS * KF)
    Xq = X.astype(ml_dtypes.float8_e3m4)
    shards = []
    for c in range(N_CORES):
        v = Xq[c * B_CORE : (c + 1) * B_CORE].reshape(B_CORE, N_SW, DCT_NBINS, KT, 128)
        p = v.transpose(1, 2, 3, 4, 0)  # [s, n, kt, p, b]
        shards.append(np.ascontiguousarray(p).reshape(NT * 128, B_CORE))
    return shards


def _chunk_plan():
    """(t_start, n_t) DMA chunks over the 64-tile stream. Small chunks at the
    front (early first matmul), 4-tile chunks through the middle, short taper
    at the tail (less serial work after the final DMA)."""
    plan = [(0, 2), (2, 2)]
    plan += [(t, 4) for t in range(4, 60, 4)]
    plan += [(60, 2), (62, 1), (63, 1)]
    return plan


def _build_program():
    nc = bacc.Bacc("TRN2", target_bir_lowering=False, debug=False, num_devices=N_CORES)
    f32 = mybir.dt.float32
    bf16 = mybir.dt.bfloat16
    f8 = mybir.dt.float8e3

    x_ap = nc.dram_tensor("x", [NT * 128, B_CORE], f8, kind="ExternalInput").ap()
    w_ap = nc.dram_tensor("w", [128, N_SW * KT * KOUT], bf16, kind="ExternalInput").ap()
    b_ap = nc.dram_tensor("bvec", [1, N_SW * KOUT + B_CORE], bf16, kind="ExternalInput").ap()
    # output stays transposed [s*128+k, b] in bf16; host upcasts + un-transposes
    out_ap = nc.dram_tensor("out", [N_SW * KOUT, B_CORE], bf16, kind="ExternalOutput").ap()

    N_WARMUP = 13

    with tile.TileContext(nc) as tc, ExitStack() as ctx:
        const = ctx.enter_context(tc.tile_pool(name="const", bufs=1))
        x_pool = ctx.enter_context(tc.tile_pool(name="xp", bufs=10))
        osb_pool = ctx.enter_context(tc.tile_pool(name="osb", bufs=4))
        pout_pool = ctx.enter_context(tc.tile_pool(name="pout", bufs=1, space="PSUM"))

        # ones row for PE warm-up matmuls (burn the p-state ramp while the
        # first DMAs are in flight); small memset so it's ready ~1.2us in
        ones_sb = const.tile([1, 384], bf16)
        nc.vector.memset(ones_sb[:], 1.0)

        x_re = x_ap.rearrange("(t p) b -> p t b", p=128)  # [128, 64, 512]

        # weights via Pool SWDGE (own descriptor engine -> keeps the HWDGE
        # queues free for the x stream), in SEPARATE tiles per chunk so
        # readers of one chunk never wait on DMAs of the others
        # (tile-granularity dependency tracking). Pool's ~1us serial preps
        # self-stagger the four transfers across the early stream; each lands
        # comfortably before its first consuming matmul. bvec last (needed
        # only by the mid-group bias matmul).
        wch = [const.tile([128, 1024], bf16, name=f"wch_{i}") for i in range(4)]
        for i in range(4):
            nc.gpsimd.dma_start(out=wch[i][:], in_=w_ap[:, bass.ts(i, 1024)])

        # bvec via SP's HWDGE (tiny transfer): ready early so the
        # mid-group bias matmul never bubbles the in-order PE queue
        bvec_sb = const.tile([1, N_SW * KOUT + B_CORE], bf16)
        nc.sync.dma_start(out=bvec_sb[:], in_=b_ap[:])

        def w_tile(s, kt):
            return wch[s * 2 + kt // 8][:, bass.ts(kt % 8, 128)]

        psums = {}
        for s in range(N_SW):
            psums[s] = pout_pool.tile([KOUT, B_CORE], f32, name=f"psum_{s}")
        scratch = pout_pool.tile([KOUT, B_CORE], f32, name="psum_warm")

        for i in range(N_WARMUP):
            nc.tensor.matmul(
                scratch[:, 0:256],
                lhsT=ones_sb[0:1, 0:KOUT],
                rhs=ones_sb[0:1, KOUT:],
                start=True,
                stop=True,
            )

        def issue_epilogue(s):
            # exact LeakyReLU (bias already seeded in PSUM), obeying the
            # one-PSUM-operand-per-instruction HW rule:
            #   r   = (psum max 0) * (1-SLOPE)         [dual-op tensor_scalar]
            #   out = (psum * SLOPE) + r               [scalar_tensor_tensor]
            r = osb_pool.tile([KOUT, B_CORE], f32, tag="r", name=f"r_{s}")
            nc.vector.tensor_scalar(
                r[:],
                psums[s][:],
                0.0,
                1.0 - SLOPE,
                mybir.AluOpType.max,
                mybir.AluOpType.mult,
            )
            osb = osb_pool.tile([KOUT, B_CORE], bf16, tag="osb", name=f"osb_{s}")
            nc.vector.scalar_tensor_tensor(
                osb[:],
                psums[s][:],
                SLOPE,
                r[:],
                mybir.AluOpType.mult,
                mybir.AluOpType.add,
            )
            # s0's out mid-stream via Pool SWDGE (keeps SP/ACT free for x);
            # s1's out at the tail via SP (idle by then, fastest DGE)
            eng = nc.gpsimd if s == 0 else nc.sync
            eng.dma_start(out=out_ap[bass.ts(s, KOUT), :], in_=osb[:])

        chunks = _chunk_plan()
        for g, (t0, nt) in enumerate(chunks):
            xab = x_pool.tile([128, 4, B_CORE], f8, tag="xab", name=f"xab_{g}")
            dma_eng = nc.sync if g % 2 == 0 else nc.scalar
            dma_eng.dma_start(out=xab[:, 0:nt, :], in_=x_re[:, bass.ds(t0, nt), :])
            for j in range(nt):
                t = t0 + j
                s, n, kt = t // (DCT_NBINS * KT), (t // KT) % DCT_NBINS, t % KT
                nc.tensor.matmul(
                    psums[s][:],
                    lhsT=w_tile(s, kt),
                    rhs=xab[:, j, :],
                    start=(n == 0 and kt == 0),
                    stop=(n == DCT_NBINS - 1 and kt == KT - 1),
                )
                if n == 0 and kt == 12:
                    # bias seeding rides mid-group (any accumulation order
                    # works); bvec arrives ~7us in via Pool SWDGE
                    nc.tensor.matmul(
                        psums[s][:],
                        lhsT=bvec_sb[0:1, bass.ts(s, KOUT)],
                        rhs=bvec_sb[0:1, N_SW * KOUT :],
                        start=False,
                        stop=False,
                    )
                if n == DCT_NBINS - 1 and kt == KT - 1:
                    issue_epilogue(s)

    nc.compile()
    return nc


def _get_program():
    if "nc" not in _CACHE:
        _CACHE["nc"] = _build_program()
    return _CACHE["nc"]


def kernel(x, conv_w, conv_b):
    global LAST_RESULT
    shards = _shard_x(x)
    w_host, bvec = _fold_weights(conv_w, conv_b)

    nc = _get_program()
    in_maps = [{"x": shards[c], "w": w_host, "bvec": bvec} for c in range(N_CORES)]
    trace = bool(int(os.environ.get("DCT_TRACE", "0")))
    res = run_bass_kernel_spmd(nc, in_maps, list(range(N_CORES)), trace=trace)
    LAST_RESULT = res
    # per-core output is [s*128+k, b] bf16; upcast + un-transpose during gather
    out = np.concatenate(
        [
            np.ascontiguousarray(res.results[c]["out"].astype(np.float32).T)
            for c in range(N_CORES)
        ],
        axis=0,
    )
    return out
